# revision 15
# baseline (speedup 1.0000x reference)
"""Trainium2 Bass kernel for EventDiffusion GNN (GCNConv + GATConv, 2 layers).

Gather-free dense-push design:
  - Layer 1 (GCN): X@W1 replicated (bf16), segment-sum as push-mode matmuls
    against host-precomputed bf16 coefficient blocks m1[b][s][g][d].
  - H^T exchanged via bf16 AllGather.
  - Layer 2 (GAT): per-core table [ones | H@W2 | v1.H] resident in SBUF.
    For each src block g the full dense alpha field over this core's 1280
    dst columns is built as  exp(leaky(L1[s] + L2[d]) + M[s,d])  where
    M in {0, ln k, -1000} is a host-streamed fp16 mask (-1000 kills
    non-edges exactly after exp; ln k handles duplicate edges exactly).
    L2 broadcast rows are built on-device from the core's own H^T slice.
    The push is 10 dense matmuls per src block into persistent PSUM
    accumulators; denominators ride along as a leading ones column (4
    blocks) or N=1 matmuls (6 blocks packed 2-per-bank).
"""

import numpy as np
import ml_dtypes

import concourse.bass as bass
import concourse.bacc as bacc
import concourse.mybir as mybir
import concourse.tile as tile
from concourse.bass_utils import run_bass_kernel_spmd

FP32 = mybir.dt.float32
BF16 = mybir.dt.bfloat16
FP16 = mybir.dt.float16

NP_BF16 = ml_dtypes.bfloat16

N_CORES = 8
D = 256
N = 10000
NPAD = 10240          # multiple of 128*8
PER = NPAD // N_CORES  # 1280
NBLK = PER // 128      # 10 dst blocks per core
NGB = NPAD // 128      # 80 src blocks
TCOLS = 260            # table cols: 0=ones | 1:257 feats | 257=L1 | pad
MNEG = -1000.0

AD = mybir.AluOpType.add
MU = mybir.AluOpType.mult
MX = mybir.AluOpType.max


# ----------------------------------------------------------------------------
# host-side preprocessing
# ----------------------------------------------------------------------------

def _prep(event_emb, edge_index, W1, b1, W2, att_src, att_dst, b2):
    X = np.ascontiguousarray(np.asarray(event_emb, np.float32))
    n = X.shape[0]
    assert n == N

    ei = np.asarray(edge_index, np.int64)
    src = np.concatenate([ei[0], np.arange(n, dtype=np.int64)])
    dst = np.concatenate([ei[1], np.arange(n, dtype=np.int64)])
    deg = np.bincount(dst, minlength=n).astype(np.float32)
    dinv = np.where(deg > 0, 1.0 / np.sqrt(deg), 0.0).astype(np.float32)
    coeff = (dinv[src] * dinv[dst]).astype(np.float32)

    core_of = dst // PER
    per_core = []
    for c in range(N_CORES):
        m = core_of == c
        s_, d_, co = src[m], dst[m], coeff[m]
        dl = d_ - c * PER
        # GCN push blocks: m1[b][s][g][dcol]
        m1 = np.zeros((NBLK, 128, NGB, 128), np.float32)
        np.add.at(m1, (dl // 128, s_ % 128, s_ // 128, dl % 128), co)
        # GAT mask: mneg[g][s][dcol] = ln(k) for k-multiplicity edge, else -1000
        cnt = np.zeros((NGB, 128, PER), np.float32)
        np.add.at(cnt, (s_ // 128, s_ % 128, dl), 1.0)
        mneg = np.where(cnt > 0, np.log(np.maximum(cnt, 1.0)), MNEG)
        # interleave pairs of g for [40, 128, 2*1280] DMA tiles
        mneg = np.ascontiguousarray(
            mneg.reshape(NGB // 2, 2, 128, PER).transpose(0, 2, 1, 3)
            .reshape(NGB // 2, 128, 2 * PER).astype(np.float16)
        )
        per_core.append(dict(
            m1=np.ascontiguousarray(m1.reshape(NBLK, 128, NGB * 128)
                                    .astype(NP_BF16)),
            mneg=mneg,
        ))

    W1 = np.asarray(W1, np.float32)
    W2 = np.asarray(W2, np.float32)
    v1 = (W2 @ np.asarray(att_src, np.float32)).astype(np.float32)
    v2 = (W2 @ np.asarray(att_dst, np.float32)).astype(np.float32)

    Xp = np.zeros((NPAD, D), np.float32)
    Xp[:n] = X
    xt = np.ascontiguousarray(Xp.T.reshape(2, 128, NPAD).astype(NP_BF16))

    w1k = np.ascontiguousarray(W1.reshape(2, 128, D).astype(NP_BF16))
    W2p = np.zeros((D, 257), np.float32)
    W2p[:, :D] = W2
    W2p[:, 256] = v1
    w2k = np.ascontiguousarray(W2p.reshape(2, 128, 257).astype(NP_BF16))
    v2k = np.ascontiguousarray(v2.reshape(2, 128, 1).astype(NP_BF16))

    row0 = np.zeros((128, 128), np.float32)
    row0[0, :] = 1.0

    shared = dict(
        xt=xt, w1=w1k, w2p=w2k, v2c=v2k,
        b1b=np.ascontiguousarray(
            np.tile(np.asarray(b1, np.float32)[None, :], (128, 1))),
        b2b=np.ascontiguousarray(
            np.tile(np.asarray(b2, np.float32)[None, :], (128, 1))),
        row0=row0,
        ident=np.eye(128, dtype=np.float32).astype(NP_BF16),
    )
    return shared, per_core, n


# ----------------------------------------------------------------------------
# device program
# ----------------------------------------------------------------------------

def _build_nc():
    nc = bacc.Bacc(
        "TRN2", target_bir_lowering=False, debug=False, num_devices=N_CORES
    )

    xt_d = nc.dram_tensor("xt", [2, 128, NPAD], BF16, kind="ExternalInput")
    w1_d = nc.dram_tensor("w1", [2, 128, D], BF16, kind="ExternalInput")
    w2_d = nc.dram_tensor("w2p", [2, 128, 257], BF16, kind="ExternalInput")
    v2_d = nc.dram_tensor("v2c", [2, 128, 1], BF16, kind="ExternalInput")
    b1_d = nc.dram_tensor("b1b", [128, D], FP32, kind="ExternalInput")
    b2_d = nc.dram_tensor("b2b", [128, D], FP32, kind="ExternalInput")
    row0_d = nc.dram_tensor("row0", [128, 128], FP32, kind="ExternalInput")
    ident_d = nc.dram_tensor("ident", [128, 128], BF16, kind="ExternalInput")
    m1_d = nc.dram_tensor("m1", [NBLK, 128, NGB * 128], BF16,
                          kind="ExternalInput")
    mneg_d = nc.dram_tensor("mneg", [NGB // 2, 128, 2 * PER], FP16,
                            kind="ExternalInput")
    out_d = nc.dram_tensor("out_slice", [PER, D], FP32, kind="ExternalOutput")

    HHALF = PER // 2
    ht_sliceA = nc.dram_tensor("ht_sliceA", [2, 128, HHALF], BF16)
    ht_sliceB = nc.dram_tensor("ht_sliceB", [2, 128, HHALF], BF16)
    ht_fullA = nc.dram_tensor("ht_fullA", [N_CORES, 2, 128, HHALF], BF16,
                              addr_space="Shared")
    ht_fullB = nc.dram_tensor("ht_fullB", [N_CORES, 2, 128, HHALF], BF16,
                              addr_space="Shared")

    with tile.TileContext(nc) as tc:
        with tc.tile_pool(name="const", bufs=1) as cp:
            w1_sb = cp.tile([128, 2, D], BF16)
            w2_sb = cp.tile([128, 2, 257], BF16)
            v2_sb = cp.tile([128, 2, 1], BF16)
            for k in range(2):
                nc.sync.dma_start(w1_sb[:, k, :], w1_d[k])
                nc.sync.dma_start(w2_sb[:, k, :], w2_d[k])
                nc.sync.dma_start(v2_sb[:, k, :], v2_d[k])
            b1_sb = cp.tile([128, D], FP32)
            nc.sync.dma_start(b1_sb[:], b1_d[:, :])
            b2_sb = cp.tile([128, D], FP32)
            nc.sync.dma_start(b2_sb[:], b2_d[:, :])
            row0_sb = cp.tile([128, 128], FP32)
            nc.sync.dma_start(row0_sb[:], row0_d[:, :])
            ident_sb = cp.tile([128, 128], BF16)
            nc.sync.dma_start(ident_sb[:], ident_d[:, :])

            # long-lived cross-phase tensors
            tbl_sb = cp.tile([128, NGB, TCOLS], BF16)   # ones|feats|L1
            lc_sb = cp.tile([128, NGB], FP32)           # L1 scalar cols
            bcl2_sb = cp.tile([128, PER], FP16)         # L2 broadcast rows
            ht_st = cp.tile([128, 2, PER], BF16)        # own H^T slice
            l2r0 = cp.tile([128, 3, 512], FP32)         # partition-0 rows

            nc.vector.memset(tbl_sb[:, :, 0:1], 1.0)
            nc.vector.memset(l2r0[:], 0.0)

            # ---------------- phase 1A: XW1 (replicated) -------------------
            half = NPAD // 2
            with (
                tc.tile_pool(name="xw1_p", bufs=1) as xwp,
                tc.tile_pool(name="xt_p", bufs=1) as xp,
                tc.tile_pool(name="m1s_p", bufs=3) as mp,
                tc.tile_pool(name="h_p", bufs=2) as hp,
                tc.psum_pool(name="ps1_p", bufs=2) as pp,
                tc.psum_pool(name="ps2_p", bufs=2) as pa,
                tc.psum_pool(name="pt_p", bufs=2) as pt,
            ):
                xw1_sb = xwp.tile([128, NGB, D], BF16)
                for hh in range(2):
                    xt_sb = xp.tile([128, 2, half], BF16, tag="xt")
                    for k in range(2):
                        nc.sync.dma_start(
                            xt_sb[:, k, :], xt_d[k, :, hh * half:(hh + 1) * half]
                        )
                    for j in range(half // 128):
                        g = hh * (half // 128) + j
                        ps = pp.tile([128, D], FP32, tag="ps1")
                        for k in range(2):
                            nc.tensor.matmul(
                                ps[:],
                                lhsT=xt_sb[:, k, j * 128:(j + 1) * 128],
                                rhs=w1_sb[:, k, :],
                                start=(k == 0),
                                stop=(k == 1),
                            )
                        nc.vector.tensor_copy(xw1_sb[:, g, :], ps[:])

                # ------------ phase 1B: GCN push aggregate + H^T -----------
                GC = 16
                for b in range(NBLK):
                    psa = pa.tile([128, D], FP32, tag="agg1")
                    for gg in range(0, NGB, GC):
                        mt = mp.tile([128, GC * 128], BF16, tag="m1s")
                        nc.sync.dma_start(
                            mt[:], m1_d[b, :, gg * 128:(gg + GC) * 128]
                        )
                        for j in range(GC):
                            g = gg + j
                            nc.tensor.matmul(
                                psa[:],
                                lhsT=mt[:, j * 128:(j + 1) * 128],
                                rhs=xw1_sb[:, g, :],
                                start=(g == 0),
                                stop=(g == NGB - 1),
                            )
                    hf = hp.tile([128, D], FP32, tag="hf")
                    nc.vector.tensor_tensor(hf[:], psa[:], b1_sb[:], op=AD)
                    hs = hp.tile([128, D], BF16, tag="hs")
                    nc.vector.tensor_scalar_max(hs[:], hf[:], 0.0)
                    for k in range(2):
                        ptt = pt.tile([128, 128], BF16, tag="pt")
                        nc.tensor.transpose(
                            ptt[:], hs[:, k * 128:(k + 1) * 128], ident_sb[:]
                        )
                        nc.vector.tensor_copy(
                            ht_st[:, k, b * 128:(b + 1) * 128], ptt[:]
                        )
                    if b == NBLK // 2 - 1:
                        # first half of the slice is done: overlap its
                        # AllGather with the remaining blocks' compute
                        for k in range(2):
                            nc.sync.dma_start(
                                ht_sliceA[k], ht_st[:, k, 0:HHALF]
                            )
                        nc.gpsimd.collective_compute(
                            "AllGather",
                            mybir.AluOpType.bypass,
                            replica_groups=[list(range(N_CORES))],
                            ins=[ht_sliceA[:, :, :]],
                            outs=[ht_fullA[:, :, :, :]],
                        )
                for k in range(2):
                    nc.sync.dma_start(ht_sliceB[k], ht_st[:, k, HHALF:PER])

            nc.gpsimd.collective_compute(
                "AllGather",
                mybir.AluOpType.bypass,
                replica_groups=[list(range(N_CORES))],
                ins=[ht_sliceB[:, :, :]],
                outs=[ht_fullB[:, :, :, :]],
            )

            # -------- BCL2: broadcast of v2.H over own dst columns ---------
            with tc.psum_pool(name="l2_p", bufs=3) as pl2:
                for ch in range(3):
                    w = 512 if ch < 2 else PER - 1024
                    psr = pl2.tile([1, 512], FP32, tag="l2r")
                    for k in range(2):
                        nc.tensor.matmul(
                            psr[0:1, 0:w],
                            lhsT=v2_sb[:, k, :],
                            rhs=ht_st[:, k, ch * 512:ch * 512 + w],
                            start=(k == 0),
                            stop=(k == 1),
                        )
                    nc.vector.tensor_copy(l2r0[0:1, ch, 0:w], psr[0:1, 0:w])
                for ch in range(3):
                    w = 512 if ch < 2 else PER - 1024
                    psb = pl2.tile([128, 512], FP32, tag="l2b")
                    nc.tensor.matmul(
                        psb[:, 0:w],
                        lhsT=row0_sb[:],
                        rhs=l2r0[:, ch, 0:w],
                        start=True,
                        stop=True,
                    )
                    nc.vector.tensor_copy(
                        bcl2_sb[:, ch * 512:ch * 512 + w], psb[:, 0:w]
                    )

            # ---------------- phase 2A: table build ------------------------
            with (
                tc.tile_pool(name="ht2_p", bufs=1) as hp2,
                tc.psum_pool(name="ps3_p", bufs=2) as pp3,
            ):
                ht_sb = hp2.tile([128, 2 * N_CORES, PER], BF16)
                for r in range(N_CORES):
                    for k in range(2):
                        nc.sync.dma_start(
                            ht_sb[:, 2 * r + k, 0:HHALF], ht_fullA[r, k]
                        )
                        nc.sync.dma_start(
                            ht_sb[:, 2 * r + k, HHALF:PER], ht_fullB[r, k]
                        )
                # first-half-of-slice blocks first: their gathered data
                # (ht_fullA) lands earlier
                g_order = [r * NBLK + j for j in range(NBLK)
                           for r in range(N_CORES)]
                for g in g_order:
                    r, j = divmod(g, NBLK)
                    ps = pp3.tile([128, 257], FP32, tag="ps3")
                    for k in range(2):
                        nc.tensor.matmul(
                            ps[:],
                            lhsT=ht_sb[:, 2 * r + k, j * 128:(j + 1) * 128],
                            rhs=w2_sb[:, k, :],
                            start=(k == 0),
                            stop=(k == 1),
                        )
                    nc.vector.tensor_copy(tbl_sb[:, g, 1:257], ps[:, 0:256])
                    nc.vector.tensor_copy(lc_sb[:, g:g + 1], ps[:, 256:257])

            # ---------------- phase 2B: GAT dense push ---------------------
            with (
                tc.tile_pool(name="mg_p", bufs=3) as mgp,
                tc.tile_pool(name="te_p", bufs=2) as tep,
                tc.tile_pool(name="m2_p", bufs=2) as m2p,
                tc.tile_pool(name="o_p", bufs=3) as op_,
                tc.psum_pool(name="pb_p", bufs=1) as pb,
                tc.psum_pool(name="pd_p", bufs=1) as pd,
            ):
                # persistent accumulators: 4 blocks with ones col, 3 packed
                # pairs for blocks 4..9, one dens tile for their denominators
                ps257 = [pb.tile([128, 257], FP32, tag=f"ps257_{i}",
                                 name=f"ps257_{i}") for i in range(4)]
                pspair = [pb.tile([128, 512], FP32, tag=f"pspair_{i}",
                                  name=f"pspair_{i}") for i in range(3)]
                dens = pd.tile([128, 12], FP32, tag="dens")

                for gp in range(NGB // 2):
                    mg = mgp.tile([128, 2 * PER], FP16, tag="mg")
                    nc.sync.dma_start(mg[:], mneg_d[gp, :, :])
                    te = tep.tile([128, 2 * PER], FP16, tag="te")
                    for j in range(2):
                        g = 2 * gp + j
                        sl = te[:, j * PER:(j + 1) * PER]
                        # leaky on DVE (STT unsupported on Pool); mask add
                        # offloaded to the otherwise-idle GpSimd engine
                        nc.vector.tensor_scalar(
                            sl, bcl2_sb[:, :], lc_sb[:, g:g + 1], None, op0=AD
                        )
                        nc.vector.scalar_tensor_tensor(
                            sl, sl, 0.2, sl, op0=MU, op1=MX
                        )
                        nc.gpsimd.tensor_tensor(
                            sl, sl, mg[:, j * PER:(j + 1) * PER], op=AD
                        )
                    m2 = m2p.tile([128, 2 * PER], BF16, tag="m2")
                    nc.scalar.activation(
                        m2[:], te[:], mybir.ActivationFunctionType.Exp
                    )
                    for j in range(2):
                        g = 2 * gp + j
                        st = (g == 0)
                        sp = (g == NGB - 1)
                        for b in range(4):
                            nc.tensor.matmul(
                                ps257[b][:],
                                lhsT=m2[:, j * PER + b * 128:
                                        j * PER + (b + 1) * 128],
                                rhs=tbl_sb[:, g, 0:257],
                                start=st, stop=sp,
                            )
                        for b in range(4, NBLK):
                            i, h_ = divmod(b - 4, 2)
                            lhs = m2[:, j * PER + b * 128:
                                     j * PER + (b + 1) * 128]
                            # start=True clears the ENTIRE psum bank's
                            # has_written bits, so only the first group
                            # touching a shared bank may carry it.
                            nc.tensor.matmul(
                                pspair[i][:, h_ * 256:(h_ + 1) * 256],
                                lhsT=lhs,
                                rhs=tbl_sb[:, g, 1:257],
                                start=(st and h_ == 0), stop=sp,
                                skip_group_check=True,
                            )
                            nc.tensor.matmul(
                                dens[:, 2 * (b - 4):2 * (b - 4) + 2],
                                lhsT=lhs,
                                rhs=tbl_sb[:, g, 0:2],
                                start=(st and b == 4), stop=sp,
                                skip_group_check=True,
                            )

                # normalize + bias + relu + store
                for b in range(NBLK):
                    if b < 4:
                        dcol = ps257[b][:, 0:1]
                        feats = ps257[b][:, 1:257]
                    else:
                        i, h_ = divmod(b - 4, 2)
                        dcol = dens[:, 2 * (b - 4):2 * (b - 4) + 1]
                        feats = pspair[i][:, h_ * 256:(h_ + 1) * 256]
                    de = op_.tile([128, 1], FP32, tag="de")
                    nc.vector.tensor_scalar_add(de[:], dcol, 1e-16)
                    rc = op_.tile([128, 1], FP32, tag="rc")
                    nc.vector.reciprocal(rc[:], de[:])
                    ob = op_.tile([128, D], FP32, tag="ob")
                    nc.vector.scalar_tensor_tensor(
                        ob[:], feats, rc[:], b2_sb[:], op0=MU, op1=AD
                    )
                    nc.vector.tensor_scalar_max(ob[:], ob[:], 0.0)
                    nc.sync.dma_start(out_d[b * 128:(b + 1) * 128, :], ob[:])
    nc.finalize()
    return nc


# ----------------------------------------------------------------------------
# entry point
# ----------------------------------------------------------------------------

_CACHE = {}


def _get_nc():
    if "nc" not in _CACHE:
        _CACHE["nc"] = _build_nc()
    return _CACHE["nc"]


def kernel(event_emb, edge_index, W1, b1, W2, att_src, att_dst, b2,
           _want_results=False, _trace=False):
    shared, per_core, n = _prep(
        event_emb, edge_index, W1, b1, W2, att_src, att_dst, b2
    )
    nc = _get_nc()
    in_maps = [{**shared, **per_core[c]} for c in range(N_CORES)]
    res = run_bass_kernel_spmd(
        nc, in_maps, core_ids=list(range(N_CORES)), trace=_trace
    )
    out = np.concatenate(
        [np.asarray(res.results[c]["out_slice"]) for c in range(N_CORES)],
        axis=0,
    )[:n]
    if _want_results:
        return out, res
    return out


# revision 30
# speedup vs baseline: 1.0936x; 1.0936x over previous
"""Trainium2 Bass kernel for EventDiffusion GNN (GCNConv + GATConv, 2 layers).

Gather-free dense-push design:
  - Layer 1 (GCN): X@W1 replicated (bf16), segment-sum as push-mode matmuls
    against host-precomputed bf16 coefficient blocks m1[b][s][g][d].
  - H^T exchanged via bf16 AllGather.
  - Layer 2 (GAT): per-core table [ones | H@W2 | v1.H] resident in SBUF.
    For each src block g the full dense alpha field over this core's 1280
    dst columns is built as  exp(leaky(L1[s] + L2[d]) + M[s,d])  where
    M in {0, ln k, -1000} is a host-streamed fp16 mask (-1000 kills
    non-edges exactly after exp; ln k handles duplicate edges exactly).
    L2 broadcast rows are built on-device from the core's own H^T slice.
    The push is 10 dense matmuls per src block into persistent PSUM
    accumulators; denominators ride along as a leading ones column (4
    blocks) or N=1 matmuls (6 blocks packed 2-per-bank).
"""

import numpy as np
import ml_dtypes

import concourse.bass as bass
import concourse.bacc as bacc
import concourse.mybir as mybir
import concourse.tile as tile
from concourse.bass_utils import run_bass_kernel_spmd

FP32 = mybir.dt.float32
BF16 = mybir.dt.bfloat16
FP16 = mybir.dt.float16

NP_BF16 = ml_dtypes.bfloat16

N_CORES = 8
D = 256
N = 10000
NPAD = 10240          # multiple of 128*8
PER = NPAD // N_CORES  # 1280
NBLK = PER // 128      # 10 dst blocks per core
NGB = NPAD // 128      # 80 src blocks
TCOLS = 260            # table cols: 0=ones | 1:257 feats | 257=L1 | pad
MNEG = -1000.0

AD = mybir.AluOpType.add
MU = mybir.AluOpType.mult
MX = mybir.AluOpType.max


# ----------------------------------------------------------------------------
# host-side preprocessing
# ----------------------------------------------------------------------------

def _prep(event_emb, edge_index, W1, b1, W2, att_src, att_dst, b2):
    X = np.ascontiguousarray(np.asarray(event_emb, np.float32))
    n = X.shape[0]
    assert n == N

    ei = np.asarray(edge_index, np.int64)
    src = np.concatenate([ei[0], np.arange(n, dtype=np.int64)])
    dst = np.concatenate([ei[1], np.arange(n, dtype=np.int64)])
    deg = np.bincount(dst, minlength=n).astype(np.float32)
    dinv = np.where(deg > 0, 1.0 / np.sqrt(deg), 0.0).astype(np.float32)
    coeff = (dinv[src] * dinv[dst]).astype(np.float32)

    core_of = dst // PER
    per_core = []
    for c in range(N_CORES):
        m = core_of == c
        s_, d_, co = src[m], dst[m], coeff[m]
        dl = d_ - c * PER
        # GCN push blocks, flipped: m1[g][s][dcol] (dst columns)
        m1 = np.zeros((NGB, 128, PER), np.float32)
        np.add.at(m1, (s_ // 128, s_ % 128, dl), co)
        # GAT mask: mneg[g][s][dcol] = ln(k) for k-multiplicity edge, else -1000
        cnt = np.zeros((NGB, 128, PER), np.float32)
        np.add.at(cnt, (s_ // 128, s_ % 128, dl), 1.0)
        mneg = np.where(cnt > 0, np.log(np.maximum(cnt, 1.0)), MNEG)
        # interleave pairs of g for [40, 128, 2*1280] DMA tiles
        mneg = np.ascontiguousarray(
            mneg.reshape(NGB // 2, 2, 128, PER).transpose(0, 2, 1, 3)
            .reshape(NGB // 2, 128, 2 * PER).astype(np.float16)
        )
        m1h = m1.astype(NP_BF16)
        per_core.append(dict(
            m1a=np.ascontiguousarray(m1h[:, :, 0:512]),
            m1b=np.ascontiguousarray(m1h[:, :, 512:1024]),
            m1c=np.ascontiguousarray(m1h[:, :, 1024:1280]),
            mneg=mneg,
        ))

    W1 = np.asarray(W1, np.float32)
    W2 = np.asarray(W2, np.float32)
    v1 = (W2 @ np.asarray(att_src, np.float32)).astype(np.float32)
    v2 = (W2 @ np.asarray(att_dst, np.float32)).astype(np.float32)

    Xp = np.zeros((NPAD, D), np.float32)
    Xp[:n] = X
    xt = np.ascontiguousarray(Xp.T.reshape(2, 128, NPAD).astype(NP_BF16))

    w1k = np.ascontiguousarray(W1.reshape(2, 128, D).astype(NP_BF16))
    b1c = np.ascontiguousarray(np.asarray(b1, np.float32).reshape(2, 128, 1))
    W2p = np.zeros((D, 257), np.float32)
    W2p[:, :D] = W2
    W2p[:, 256] = v1
    w2k = np.ascontiguousarray(W2p.reshape(2, 128, 257).astype(NP_BF16))
    v2k = np.ascontiguousarray(v2.reshape(2, 128, 1).astype(NP_BF16))

    row0 = np.zeros((128, 128), np.float32)
    row0[0, :] = 1.0

    shared = dict(
        xt=xt, w1=w1k, w2p=w2k, v2c=v2k,
        b1c=b1c,
        b2b=np.ascontiguousarray(
            np.tile(np.asarray(b2, np.float32)[None, :], (128, 1))),
        row0=row0,
    )
    return shared, per_core, n


# ----------------------------------------------------------------------------
# device program
# ----------------------------------------------------------------------------

def _build_nc():
    nc = bacc.Bacc(
        "TRN2", target_bir_lowering=False, debug=False, num_devices=N_CORES
    )

    xt_d = nc.dram_tensor("xt", [2, 128, NPAD], BF16, kind="ExternalInput")
    w1_d = nc.dram_tensor("w1", [2, 128, D], BF16, kind="ExternalInput")
    w2_d = nc.dram_tensor("w2p", [2, 128, 257], BF16, kind="ExternalInput")
    v2_d = nc.dram_tensor("v2c", [2, 128, 1], BF16, kind="ExternalInput")
    b1_d = nc.dram_tensor("b1c", [2, 128, 1], FP32, kind="ExternalInput")
    b2_d = nc.dram_tensor("b2b", [128, D], FP32, kind="ExternalInput")
    row0_d = nc.dram_tensor("row0", [128, 128], FP32, kind="ExternalInput")
    m1a_d = nc.dram_tensor("m1a", [NGB, 128, 512], BF16, kind="ExternalInput")
    m1b_d = nc.dram_tensor("m1b", [NGB, 128, 512], BF16, kind="ExternalInput")
    m1c_d = nc.dram_tensor("m1c", [NGB, 128, 256], BF16, kind="ExternalInput")
    mneg_d = nc.dram_tensor("mneg", [NGB // 2, 128, 2 * PER], FP16,
                            kind="ExternalInput")
    out_d = nc.dram_tensor("out_slice", [PER, D], FP32, kind="ExternalOutput")

    HHALF = 512
    ht_sliceA = nc.dram_tensor("ht_sliceA", [2, 128, HHALF], BF16)
    ht_sliceB = nc.dram_tensor("ht_sliceB", [2, 128, PER - HHALF], BF16)
    ht_fullA = nc.dram_tensor("ht_fullA", [N_CORES, 2, 128, HHALF], BF16,
                              addr_space="Shared")
    ht_fullB = nc.dram_tensor("ht_fullB", [N_CORES, 2, 128, PER - HHALF],
                              BF16, addr_space="Shared")

    with tile.TileContext(nc) as tc:
        with tc.tile_pool(name="const", bufs=1) as cp:
            w1_sb = cp.tile([128, 2, D], BF16)
            w2_sb = cp.tile([128, 2, 257], BF16)
            v2_sb = cp.tile([128, 2, 1], BF16)
            for k in range(2):
                nc.sync.dma_start(w1_sb[:, k, :], w1_d[k])
                nc.sync.dma_start(w2_sb[:, k, :], w2_d[k])
                nc.sync.dma_start(v2_sb[:, k, :], v2_d[k])
            b1c_sb = cp.tile([128, 2], FP32)
            for k in range(2):
                nc.sync.dma_start(b1c_sb[:, k:k + 1], b1_d[k])
            b2_sb = cp.tile([128, D], FP32)
            nc.sync.dma_start(b2_sb[:], b2_d[:, :])
            row0_sb = cp.tile([128, 128], FP32)
            nc.sync.dma_start(row0_sb[:], row0_d[:, :])

            # long-lived cross-phase tensors
            tbl_sb = cp.tile([128, NGB, TCOLS], BF16)   # ones|feats|L1
            lc_sb = cp.tile([128, NGB], FP32)           # L1 scalar cols
            bcl2_sb = cp.tile([128, PER], FP16)         # L2 broadcast rows
            ht_st = cp.tile([128, 2, PER], BF16)        # own H^T slice
            l2r0 = cp.tile([128, 3, 512], FP32)         # partition-0 rows

            nc.vector.memset(tbl_sb[:, :, 0:1], 1.0)
            nc.vector.memset(l2r0[:], 0.0)

            # ---------------- phase 1A: XW1 (replicated) -------------------
            half = NPAD // 2
            with (
                tc.tile_pool(name="xw1_p", bufs=1) as xwp,
                tc.tile_pool(name="xt_p", bufs=1) as xp,
                tc.tile_pool(name="m1s_p", bufs=4) as mp,
                tc.psum_pool(name="ps1_p", bufs=2) as pp,
                tc.psum_pool(name="ps2_p", bufs=1) as pa,
            ):
                xw1_sb = xwp.tile([128, NGB, D], BF16)
                for hh in range(2):
                    xt_sb = xp.tile([128, 2, half], BF16, tag="xt")
                    for k in range(2):
                        nc.sync.dma_start(
                            xt_sb[:, k, :], xt_d[k, :, hh * half:(hh + 1) * half]
                        )
                    for j in range(half // 128):
                        g = hh * (half // 128) + j
                        ps = pp.tile([128, D], FP32, tag="ps1")
                        for k in range(2):
                            nc.tensor.matmul(
                                ps[:],
                                lhsT=xt_sb[:, k, j * 128:(j + 1) * 128],
                                rhs=w1_sb[:, k, :],
                                start=(k == 0),
                                stop=(k == 1),
                            )
                        nc.vector.tensor_copy(xw1_sb[:, g, :], ps[:])

                # -------- phase 1B: GCN push aggregate, flipped ------------
                # psum[feat_half, dst_cols] += xw1_g_half.T @ m1_g_cols
                # produces H^T directly; three column passes (512|512|256)
                # so the first AllGather overlaps the later passes.
                m1h_d = [m1a_d, m1b_d, m1c_d]
                chunks = [(0, 512), (512, 512), (1024, 256)]
                for ch, (c0, cw) in enumerate(chunks):
                    psa = [pa.tile([128, 512], FP32, tag=f"agg{h}",
                                   name=f"agg{ch}{h}") for h in range(2)]
                    for g in range(NGB):
                        mt = mp.tile([128, 512], BF16, tag="m1s",
                                     name=f"mt{ch}{g}")
                        nc.sync.dma_start(mt[0:128, 0:cw], m1h_d[ch][g])
                        for h in range(2):
                            nc.tensor.matmul(
                                psa[h][:, 0:cw],
                                lhsT=xw1_sb[:, g, h * 128:(h + 1) * 128],
                                rhs=mt[0:128, 0:cw],
                                start=(g == 0),
                                stop=(g == NGB - 1),
                            )
                    for h in range(2):
                        nc.vector.tensor_scalar(
                            ht_st[:, h, c0:c0 + cw], psa[h][:, 0:cw],
                            b1c_sb[:, h:h + 1], 0.0, op0=AD, op1=MX,
                        )
                    if ch == 0:
                        for k in range(2):
                            nc.sync.dma_start(
                                ht_sliceA[k], ht_st[:, k, 0:HHALF]
                            )
                        nc.gpsimd.collective_compute(
                            "AllGather",
                            mybir.AluOpType.bypass,
                            replica_groups=[list(range(N_CORES))],
                            ins=[ht_sliceA[:, :, :]],
                            outs=[ht_fullA[:, :, :, :]],
                        )
                for k in range(2):
                    nc.sync.dma_start(ht_sliceB[k], ht_st[:, k, HHALF:PER])

            nc.gpsimd.collective_compute(
                "AllGather",
                mybir.AluOpType.bypass,
                replica_groups=[list(range(N_CORES))],
                ins=[ht_sliceB[:, :, :]],
                outs=[ht_fullB[:, :, :, :]],
            )

            # -------- BCL2: broadcast of v2.H over own dst columns ---------
            with tc.psum_pool(name="l2_p", bufs=3) as pl2:
                for ch in range(3):
                    w = 512 if ch < 2 else PER - 1024
                    psr = pl2.tile([1, 512], FP32, tag="l2r")
                    for k in range(2):
                        nc.tensor.matmul(
                            psr[0:1, 0:w],
                            lhsT=v2_sb[:, k, :],
                            rhs=ht_st[:, k, ch * 512:ch * 512 + w],
                            start=(k == 0),
                            stop=(k == 1),
                        )
                    nc.vector.tensor_copy(l2r0[0:1, ch, 0:w], psr[0:1, 0:w])
                for ch in range(3):
                    w = 512 if ch < 2 else PER - 1024
                    psb = pl2.tile([128, 512], FP32, tag="l2b")
                    nc.tensor.matmul(
                        psb[:, 0:w],
                        lhsT=row0_sb[:],
                        rhs=l2r0[:, ch, 0:w],
                        start=True,
                        stop=True,
                    )
                    nc.vector.tensor_copy(
                        bcl2_sb[:, ch * 512:ch * 512 + w], psb[:, 0:w]
                    )

            # ---------------- phase 2A: table build ------------------------
            with (
                tc.tile_pool(name="ht2_p", bufs=1) as hp2,
                tc.psum_pool(name="ps3_p", bufs=2) as pp3,
            ):
                ht_sb = hp2.tile([128, 2 * N_CORES, PER], BF16)
                for r in range(N_CORES):
                    for k in range(2):
                        nc.sync.dma_start(
                            ht_sb[:, 2 * r + k, 0:HHALF], ht_fullA[r, k]
                        )
                        nc.sync.dma_start(
                            ht_sb[:, 2 * r + k, HHALF:PER], ht_fullB[r, k]
                        )
                # first-half-of-slice blocks first: their gathered data
                # (ht_fullA) lands earlier
                g_order = [r * NBLK + j for j in range(NBLK)
                           for r in range(N_CORES)]
                for g in g_order:
                    r, j = divmod(g, NBLK)
                    ps = pp3.tile([128, 257], FP32, tag="ps3")
                    for k in range(2):
                        nc.tensor.matmul(
                            ps[:],
                            lhsT=ht_sb[:, 2 * r + k, j * 128:(j + 1) * 128],
                            rhs=w2_sb[:, k, :],
                            start=(k == 0),
                            stop=(k == 1),
                        )
                    nc.vector.tensor_copy(tbl_sb[:, g, 1:257], ps[:, 0:256])
                    nc.vector.tensor_copy(lc_sb[:, g:g + 1], ps[:, 256:257])

            # ---------------- phase 2B: GAT dense push ---------------------
            with (
                tc.tile_pool(name="mg_p", bufs=3) as mgp,
                tc.tile_pool(name="te_p", bufs=2) as tep,
                tc.tile_pool(name="m2_p", bufs=2) as m2p,
                tc.tile_pool(name="o_p", bufs=3) as op_,
                tc.psum_pool(name="pb_p", bufs=1) as pb,
                tc.psum_pool(name="pd_p", bufs=1) as pd,
            ):
                # persistent accumulators: 4 blocks with ones col, 3 packed
                # pairs for blocks 4..9, one dens tile for their denominators
                ps257 = [pb.tile([128, 257], FP32, tag=f"ps257_{i}",
                                 name=f"ps257_{i}") for i in range(4)]
                pspair = [pb.tile([128, 512], FP32, tag=f"pspair_{i}",
                                  name=f"pspair_{i}") for i in range(3)]
                dens = pd.tile([128, 12], FP32, tag="dens")

                for gp in range(NGB // 2):
                    mg = mgp.tile([128, 2 * PER], FP16, tag="mg")
                    nc.sync.dma_start(mg[:], mneg_d[gp, :, :])
                    te = tep.tile([128, 2 * PER], FP16, tag="te")
                    for j in range(2):
                        g = 2 * gp + j
                        sl = te[:, j * PER:(j + 1) * PER]
                        # leaky on DVE (STT unsupported on Pool); mask add
                        # offloaded to the otherwise-idle GpSimd engine
                        nc.vector.tensor_scalar(
                            sl, bcl2_sb[:, :], lc_sb[:, g:g + 1], None, op0=AD
                        )
                        nc.vector.scalar_tensor_tensor(
                            sl, sl, 0.2, sl, op0=MU, op1=MX
                        )
                        nc.vector.tensor_tensor(
                            sl, sl, mg[:, j * PER:(j + 1) * PER], op=AD
                        )
                    m2 = m2p.tile([128, 2 * PER], BF16, tag="m2")
                    nc.scalar.activation(
                        m2[:], te[:], mybir.ActivationFunctionType.Exp
                    )
                    for j in range(2):
                        g = 2 * gp + j
                        st = (g == 0)
                        sp = (g == NGB - 1)
                        for b in range(4):
                            nc.tensor.matmul(
                                ps257[b][:],
                                lhsT=m2[:, j * PER + b * 128:
                                        j * PER + (b + 1) * 128],
                                rhs=tbl_sb[:, g, 0:257],
                                start=st, stop=sp,
                            )
                        for b in range(4, NBLK):
                            i, h_ = divmod(b - 4, 2)
                            lhs = m2[:, j * PER + b * 128:
                                     j * PER + (b + 1) * 128]
                            # start=True clears the ENTIRE psum bank's
                            # has_written bits, so only the first group
                            # touching a shared bank may carry it.
                            nc.tensor.matmul(
                                pspair[i][:, h_ * 256:(h_ + 1) * 256],
                                lhsT=lhs,
                                rhs=tbl_sb[:, g, 1:257],
                                start=(st and h_ == 0), stop=sp,
                                skip_group_check=True,
                            )
                            nc.tensor.matmul(
                                dens[:, 2 * (b - 4):2 * (b - 4) + 2],
                                lhsT=lhs,
                                rhs=tbl_sb[:, g, 0:2],
                                start=(st and b == 4), stop=sp,
                                skip_group_check=True,
                            )

                # normalize + bias + relu + store
                for b in range(NBLK):
                    if b < 4:
                        dcol = ps257[b][:, 0:1]
                        feats = ps257[b][:, 1:257]
                    else:
                        i, h_ = divmod(b - 4, 2)
                        dcol = dens[:, 2 * (b - 4):2 * (b - 4) + 1]
                        feats = pspair[i][:, h_ * 256:(h_ + 1) * 256]
                    de = op_.tile([128, 1], FP32, tag="de")
                    nc.vector.tensor_scalar_add(de[:], dcol, 1e-16)
                    rc = op_.tile([128, 1], FP32, tag="rc")
                    nc.vector.reciprocal(rc[:], de[:])
                    ob = op_.tile([128, D], FP32, tag="ob")
                    nc.vector.scalar_tensor_tensor(
                        ob[:], feats, rc[:], b2_sb[:], op0=MU, op1=AD
                    )
                    nc.vector.tensor_scalar_max(ob[:], ob[:], 0.0)
                    nc.sync.dma_start(out_d[b * 128:(b + 1) * 128, :], ob[:])
    nc.finalize()
    return nc


# ----------------------------------------------------------------------------
# entry point
# ----------------------------------------------------------------------------

_CACHE = {}


def _get_nc():
    if "nc" not in _CACHE:
        _CACHE["nc"] = _build_nc()
    return _CACHE["nc"]


def kernel(event_emb, edge_index, W1, b1, W2, att_src, att_dst, b2,
           _want_results=False, _trace=False):
    shared, per_core, n = _prep(
        event_emb, edge_index, W1, b1, W2, att_src, att_dst, b2
    )
    nc = _get_nc()
    in_maps = [{**shared, **per_core[c]} for c in range(N_CORES)]
    res = run_bass_kernel_spmd(
        nc, in_maps, core_ids=list(range(N_CORES)), trace=_trace
    )
    out = np.concatenate(
        [np.asarray(res.results[c]["out_slice"]) for c in range(N_CORES)],
        axis=0,
    )[:n]
    if _want_results:
        return out, res
    return out
